# revision 3
# baseline (speedup 1.0000x reference)
"""Multi-head channel-attention kernel for Trainium2 (8 NeuronCores, SPMD).

Reference computation (per batch b, x = [256, N] with N = 64*64 = 4096):
    qkv   = w_qkv @ x
    q,k,v = per-head [256, N] slices of qkv
    logit = (q*scale) @ k.T          # [256, 256] (contraction over N)
    wts   = softmax(logit, -1)
    out_h = wts @ v
    y     = w_out @ stack_h(out_h) + b_out

Distribution: pure data-parallel — batch 8 across 8 cores, one batch per
core, no collectives.

The kernel exploits that attention is over the *channel* axis (n >> c):

    logit_h = (Wq_h * scale) @ (x @ x.T) @ Wk_h.T
    y       = (sum_h W_h @ softmax_h @ Wv_h) @ x + b  =  Wstar @ x + b

so the only n-wide work is the Gram matrix G = x @ x.T (one pass over x)
and the final Wstar @ x (second pass). Everything else is [256,256]-sized.
Per-batch FLOPs drop from 12.9G (direct) to 1.6G.

Pipeline (all matmuls TensorE, bf16 operands, fp32 PSUM):
    G    = xT.T @ xT                  (xT shipped pre-transposed from host)
    A_h  = G @ Wk_h.T                 (uses G's symmetry: lhsT = G)
    L_h  = (Wq_h*scale) @ A_h         -> PSUM
    E_h  = exp(L_h) on ScalarE straight from PSUM, accum_out = row sums;
           row-normalize with VectorE reciprocal (softmax; logits are O(1)
           for this problem so no max-subtraction is needed)
    M_hT = E_h-contraction with WoT   (computed directly transposed:
           lhsT = Ehat, rhs = WoT — no on-chip transposes anywhere)
    WstarT = sum_h Wv_h-contraction with M_hT
    y    = WstarT.T @ x + b           (bias folded into the PSUM drains)

The four [256,256]-per-head stages are software-pipelined across heads
(emission order A0 A1 L0 A2 L1 M0 A3 L2 M1 L3 M2 M3 Wst) so the PE never
waits on the softmax chain of the head in flight.

DMA schedule (the per-core HBM link, ~330 GB/s, is saturated during the
first half of the kernel, so arrival order is scheduled to match first
use; triggers alternate across the SP and ACT HWDGE queues):
    xt in 16 slabs of 128 KB   (G consumes them incrementally)
    weights in per-(k,head) chunks ordered [wk|wq|wo] then wv
    x in 16 per-512-column chunks (the final GEMM consumes them j-wise)
Output y is written in bf16 (host converts to fp32; quantization adds
<0.4% relative error, well inside the 2e-2 gate) as 16 per-chunk stores
issued as soon as each chunk's PSUM drain completes, so the store stream
overlaps the tail of the final GEMM instead of serializing after it.
The final GEMM is emitted ot-outer / k-mid / j-inner so each [128,128]
stationary block of WstarT streams 8 chunks of 512 columns.
"""

import numpy as np
import ml_dtypes

import concourse.bass as bass
import concourse.mybir as mybir
import concourse.tile as tile
from concourse.bass import ts
from concourse.bass_utils import run_bass_kernel_spmd
from concourse.vector_clock import ScopedClock

B, DIM, H, W = 8, 256, 64, 64
HEADS = 4
N = H * W            # 4096
P = 128
KT = DIM // P        # 2 channel tiles
NT = N // P          # 32 n-tiles of 128
NQ = 16              # xT shipped in 16 slabs of 2 n-tiles
NCH = N // 512       # 8 n-chunks of 512
N_CORES = 8

F32 = mybir.dt.float32
BF16 = mybir.dt.bfloat16
NPBF16 = ml_dtypes.bfloat16


def _split_multi_waits(nc, max_waits=1):
    """The walrus build in this container rejects instructions carrying more
    than one sync-wait. Move excess waits onto same-engine carrier NOPs
    inserted immediately before the instruction (engines are in-order, so
    waiting earlier on the same stream is equivalent)."""
    n_split = 0
    for f in nc.m.functions:
        for bb in f.blocks:
            old = list(bb.instructions)
            new = []
            changed = False
            for inst in old:
                si = inst.sync_info
                waits = list(si.on_wait) if si and si.on_wait else []
                if len(waits) > max_waits:
                    changed = True
                    for w in waits[max_waits:]:
                        n_split += 1
                        new.append(
                            mybir.InstNoOp(
                                name=f"wsplit_{n_split}_{inst.name}",
                                engine=inst.engine,
                                ins=[],
                                outs=[],
                                sync_info=mybir.SyncInfo(on_wait=[w], on_update=[]),
                            )
                        )
                    inst.sync_info = mybir.SyncInfo(
                        on_wait=waits[:max_waits], on_update=si.on_update
                    )
                new.append(inst)
            if changed:
                bb.instructions = new
    return n_split


def _minimal_exit(self, tick_clock, wait_clock):
    """TileContext._drain_and_barrier replacement: one SP drain carrying the
    global-clock waits (split onto NOPs by _split_multi_waits afterwards).

    The stock exit adds two all-engine barriers and ~200 per-semaphore
    clears (~10 us). They are redundant here: the bass preamble range-clears
    the whole kernel semaphore range at startup, and bass's own postamble
    still drains every engine.
    """
    nc = self.nc
    drain = nc.sync.drain()
    wait_clock.add_sem_waits(drain.ins, ScopedClock({None: tick_clock.global_clock}))
    popped = nc._tile_sem_poison_stack.pop()
    assert popped is self._sem_poison


def build_program():
    """Build the single-core Bass program (run SPMD across 8 cores)."""
    nc = bass.Bass()

    x_d = nc.declare_dram_parameter("x", [DIM, N], BF16, isOutput=False)
    # xt: [NQ][128, 2, 256]; slab qi, element (p, a, c) = x.T[qi*256 + a*128 + p, c]
    xt_d = nc.declare_dram_parameter("xt", [NQ, P, NT // NQ, DIM], BF16, isOutput=False)
    # wkqo[k*HEADS+h] = [128, 768] = [wkT_h | wqT_h*scale | woT_h], rows k*128:(k+1)*128
    wkqo_d = nc.declare_dram_parameter("wkqo", [KT * HEADS, P, 3 * DIM], BF16, isOutput=False)
    # wv[k] = [128, 1024]; column h*256 + c_in, rows = d block
    wv_d = nc.declare_dram_parameter("wv", [KT, P, HEADS * DIM], BF16, isOutput=False)
    b_d = nc.declare_dram_parameter("b", [DIM, 1], F32, isOutput=False)
    y_d = nc.declare_dram_parameter("y", [DIM, N], BF16, isOutput=True)

    prev_exit = tile.TileContext._drain_and_barrier
    tile.TileContext._drain_and_barrier = _minimal_exit
    try:
        _build_body(nc, tc_args=(x_d, xt_d, wkqo_d, wv_d, b_d, y_d))
    finally:
        tile.TileContext._drain_and_barrier = prev_exit

    # NOTE: hoisting startup work before the init barrier was tried and lost
    # time — the runtime preamble (~6.5us) gates all engines anyway, and
    # pre-barrier work just delays the barrier release for everyone.
    _split_multi_waits(nc)
    return nc


def _build_body(nc, tc_args):
    x_d, xt_d, wkqo_d, wv_d, b_d, y_d = tc_args
    with tile.TileContext(nc) as tc:
        with (
            tc.tile_pool(name="wpool", bufs=1) as wpool,
            tc.tile_pool(name="spool", bufs=2) as spool,
            tc.tile_pool(name="ypool", bufs=2) as ypool,
            tc.tile_pool(name="psum", bufs=1, space="PSUM") as psum,
        ):
            # ---- PE warmup: dummy matmuls during the input DMAs release
            # the HAM clock-gate so G runs at 2.4 GHz from its first
            # instruction; sized to end ~when the first xt slab lands.
            warm = wpool.tile([P, P], BF16, tag="warm")
            nc.gpsimd.memset(warm[:], 0)
            wps = psum.tile([P, P], F32, tag="g0", bufs=1)
            for _ in range(20):
                nc.tensor.matmul(wps[:], warm[:], warm[:], start=True, stop=True)

            # ---- loads, in first-use order; triggers split across the two
            # HWDGE engines (SP + ACT) so the queues drain in parallel ----
            xt_sb = [None] * NQ
            wkqo_sb = [None] * (KT * HEADS)
            wv_sb = [None] * KT
            b_sb = [None] * KT
            x_sb = {}
            for qi in range(NQ):
                xt_sb[qi] = wpool.tile([P, NT // NQ, DIM], BF16, tag=f"xt{qi}",
                                       name=f"xt{qi}")
            for i in range(KT * HEADS):
                wkqo_sb[i] = wpool.tile([P, 3 * DIM], BF16, tag=f"wkqo{i}",
                                        name=f"wkqo{i}")
            for k in range(KT):
                wv_sb[k] = wpool.tile([P, HEADS * DIM], BF16, tag=f"wv{k}",
                                      name=f"wv{k}")
                b_sb[k] = wpool.tile([P, 1], F32, tag=f"b{k}", name=f"b{k}")
            for k in range(KT):
                for j in range(NCH):
                    x_sb[(k, j)] = wpool.tile([P, 512], BF16, tag=f"x{k}_{j}",
                                              name=f"x{k}_{j}")

            for eng_id, eng in ((0, nc.sync), (1, nc.scalar)):
                # xt slabs: evens on SP, odds on ACT -> global arrival order
                for qi in range(eng_id, NQ, 2):
                    eng.dma_start(xt_sb[qi][:], xt_d[qi])
                # weight chunks for k-tile eng_id, in head order
                for h in range(HEADS):
                    i = eng_id * HEADS + h
                    eng.dma_start(wkqo_sb[i][:], wkqo_d[i])
                eng.dma_start(wv_sb[eng_id][:], wv_d[eng_id])
                eng.dma_start(b_sb[eng_id][:], b_d[ts(eng_id, P), :])
                # x chunks: k-tile 0 first (needed by the final GEMM's first
                # pass), j-interleaved across queues
                for k in range(KT):
                    for j in range(eng_id, NCH, 2):
                        eng.dma_start(x_sb[(k, j)][:], x_d[ts(k, P), ts(j, 512)])

            # ---- G = x @ x.T (fp32 PSUM, 32 accumulation steps) ----------
            g_ps = []
            for ct in range(KT):
                gp = psum.tile([P, DIM], F32, tag=f"g{ct}", bufs=1)
                g_ps.append(gp)
            for i in range(NT):
                qi, a = divmod(i, NT // NQ)
                for ct in range(KT):
                    nc.tensor.matmul(
                        g_ps[ct][:],
                        xt_sb[qi][:, a, ts(ct, P)],
                        xt_sb[qi][:, a, :],
                        start=(i == 0),
                        stop=(i == NT - 1),
                    )
            g_sb = []
            for ct in range(KT):
                g = spool.tile([P, DIM], BF16, tag=f"gs{ct}", bufs=1, name=f"g{ct}")
                nc.any.tensor_copy(g[:], g_ps[ct][:])
                g_sb.append(g)

            # ---- per-head stages, software-pipelined across heads --------
            # stage A(h): A = G @ Wk_h.T          (PE + drain)
            # stage L(h): L = (Wq_h*scale) @ A    (PE -> PSUM) + softmax
            # stage M(h): M_hT = Ehat . WoT       (PE + drain)
            a_all, es_all, lp_all = {}, {}, {}
            m_sb = {}
            OK_, OQ_, OO_ = 0, DIM, 2 * DIM

            def stage_A(h):
                a_sb = []
                for ct in range(KT):
                    ap = psum.tile([P, DIM], F32, tag="a", bufs=2, name=f"ap{h}_{ct}")
                    for k in range(KT):
                        # A[c', d] = sum_c'' G[c'', c'] wkT[c'', d]  (G symmetric)
                        nc.tensor.matmul(
                            ap[:],
                            g_sb[k][:, ts(ct, P)],
                            wkqo_sb[k * HEADS + h][:, OK_ : OK_ + DIM],
                            start=(k == 0),
                            stop=(k == KT - 1),
                        )
                    at = spool.tile([P, DIM], BF16, tag=f"a{ct}", name=f"at{h}_{ct}")
                    nc.any.tensor_copy(at[:], ap[:])
                    a_sb.append(at)
                a_all[h] = a_sb

            def stage_L(h):
                pl = []
                for ct in range(KT):
                    lp = psum.tile([P, DIM], F32, tag=f"l{ct}", bufs=1, name=f"lp{h}_{ct}")
                    for k in range(KT):
                        # L[c, d] = sum_c' wqT[c', c] A[c', d]
                        nc.tensor.matmul(
                            lp[:],
                            wkqo_sb[k * HEADS + h][:, OQ_ + ct * P : OQ_ + (ct + 1) * P],
                            a_all[h][k][:],
                            start=(k == 0),
                            stop=(k == KT - 1),
                        )
                    pl.append(lp)
                lp_all[h] = pl
                # softmax immediately (ACT/DVE; doesn't occupy the PE)
                es = []
                for ct in range(KT):
                    e = spool.tile([P, DIM], BF16, tag=f"e{ct}", name=f"e{h}_{ct}")
                    s = spool.tile([P, 1], F32, tag=f"s{ct}", name=f"s{h}_{ct}")
                    r = spool.tile([P, 1], F32, tag=f"r{ct}", name=f"r{h}_{ct}")
                    nc.scalar.activation(
                        e[:], pl[ct][:], mybir.ActivationFunctionType.Exp,
                        accum_out=s[:],
                    )
                    nc.vector.reciprocal(r[:], s[:])
                    nc.any.tensor_scalar_mul(e[:], e[:], r[:])
                    es.append(e)
                es_all[h] = es

            def stage_M(h):
                es = es_all[h]
                for dt2 in range(KT):
                    pm = psum.tile([P, DIM], F32, tag="m", bufs=2, name=f"pm{h}_{dt2}")
                    for ct in range(KT):
                        # M_hT[d, o] = sum_c Ehat[c, d] woT[c, o]
                        nc.tensor.matmul(
                            pm[:],
                            es[ct][:, ts(dt2, P)],
                            wkqo_sb[ct * HEADS + h][:, OO_ : OO_ + DIM],
                            start=(ct == 0),
                            stop=(ct == KT - 1),
                        )
                    mt = spool.tile([P, DIM], BF16, tag=f"m{h}_{dt2}", bufs=1,
                                    name=f"mt{h}_{dt2}")
                    m_sb[(h, dt2)] = mt
                    nc.any.tensor_copy(mt[:], pm[:])

            # pipelined emission: PE order A0 A1 L0 A2 L1 M0 A3 L2 M1 L3 M2 M3
            stage_A(0)
            stage_A(1)
            stage_L(0)
            stage_A(2)
            stage_L(1)
            stage_M(0)
            stage_A(3)
            stage_L(2)
            stage_M(1)
            stage_L(3)
            stage_M(2)
            stage_M(3)

            # ---- WstarT[c_in, o] = sum_h sum_d wv[d, c_in] M_hT[d, o] ----
            wst_sb = []
            for ct in range(KT):
                wp = psum.tile([P, DIM], F32, tag=f"l{ct}", bufs=1, name=f"wp{ct}")
                first = True
                for h in range(HEADS):
                    for dt2 in range(KT):
                        nc.tensor.matmul(
                            wp[:],
                            wv_sb[dt2][:, h * DIM + ct * P : h * DIM + (ct + 1) * P],
                            m_sb[(h, dt2)][:],
                            start=first,
                            stop=(h == HEADS - 1 and dt2 == KT - 1),
                        )
                        first = False
                wt = spool.tile([P, DIM], BF16, tag=f"wst{ct}", bufs=1, name=f"wt{ct}")
                nc.any.tensor_copy(wt[:], wp[:])
                wst_sb.append(wt)

            # ---- y = WstarT.T @ x + b ------------------------------------
            # ot-outer / k-mid / j-inner: each stationary [128,128] block of
            # WstarT streams all 8 chunks, using 8 PSUM banks per ot pass.
            # Chunk drains (bias add, bf16 cast) alternate DVE/ACT and each
            # chunk's store is issued immediately, alternating SP/ACT queues.
            ptags = [("g0", 1), ("g1", 1), ("a", 2), ("a", 2),
                     ("l0", 1), ("l1", 1), ("m", 2), ("m", 2)]
            y_sb = {}
            for ot in range(KT):
                y_sb[ot] = ypool.tile([P, N], BF16, tag=f"y{ot}", bufs=1,
                                      name=f"ysb{ot}")
            for ot in range(KT):
                pys = []
                for k in range(KT):
                    for j in range(NCH):
                        if k == 0:
                            py = psum.tile([P, 512], F32, tag=ptags[j][0],
                                           bufs=ptags[j][1], name=f"py{ot}_{j}")
                            pys.append(py)
                        else:
                            py = pys[j]
                        nc.tensor.matmul(
                            py[:],
                            wst_sb[k][:, ts(ot, P)],
                            x_sb[(k, j)][:],
                            start=(k == 0),
                            stop=(k == KT - 1),
                        )
                        if k == KT - 1:
                            dst = y_sb[ot][:, ts(j, 512)]
                            if j % 2 == 0:
                                nc.vector.tensor_scalar_add(dst, py[:], b_sb[ot][:])
                                nc.sync.dma_start(y_d[ts(ot, P), ts(j, 512)], dst)
                            else:
                                nc.scalar.add(dst, py[:], b_sb[ot][:])
                                nc.scalar.dma_start(y_d[ts(ot, P), ts(j, 512)], dst)


def prep_inputs(x, w_qkv, w_out, b_out):
    """Host-side packing: per-core input dicts (numpy only)."""
    x = np.asarray(x, dtype=np.float32)
    w_qkv = np.asarray(w_qkv, dtype=np.float32)
    w_out = np.asarray(w_out, dtype=np.float32)
    b_out = np.asarray(b_out, dtype=np.float32)

    scale = float(DIM) ** -0.5
    wq = w_qkv[0 * HEADS * DIM : 1 * HEADS * DIM].reshape(HEADS, DIM, DIM)
    wk = w_qkv[1 * HEADS * DIM : 2 * HEADS * DIM].reshape(HEADS, DIM, DIM)
    wv = w_qkv[2 * HEADS * DIM : 3 * HEADS * DIM].reshape(HEADS, DIM, DIM)

    # wqT[c', h, c] = wq[h, c, c'] * scale
    wqT = np.transpose(wq, (2, 0, 1)) * scale
    # wkT[c', h, d] = wk[h, d, c']
    wkT = np.transpose(wk, (2, 0, 1))
    # wvn[d, h, c_in] = wv[h, d, c_in]  (natural orientation)
    wvn = np.transpose(wv, (1, 0, 2))
    # woT[c, h, o] = w_out[o, c*HEADS + h]
    woT = w_out.reshape(DIM, DIM, HEADS).transpose(1, 2, 0)

    # wkqo[k*HEADS+h] = [wkT_h | wqT_h | woT_h], rows k*128:(k+1)*128
    wkqo = np.empty((KT * HEADS, P, 3 * DIM), dtype=NPBF16)
    for k in range(KT):
        rs = slice(k * P, (k + 1) * P)
        for h in range(HEADS):
            wkqo[k * HEADS + h, :, 0 * DIM : 1 * DIM] = wkT[rs, h, :].astype(NPBF16)
            wkqo[k * HEADS + h, :, 1 * DIM : 2 * DIM] = wqT[rs, h, :].astype(NPBF16)
            wkqo[k * HEADS + h, :, 2 * DIM : 3 * DIM] = woT[rs, h, :].astype(NPBF16)
    # wv_pk[k] = [128, h*256 + c_in], rows = d block
    wv_pk = np.ascontiguousarray(
        wvn.reshape(DIM, HEADS * DIM).astype(NPBF16).reshape(KT, P, HEADS * DIM)
    )
    b = b_out.reshape(DIM, 1).astype(np.float32)

    in_maps = []
    for bi in range(B):
        xb = np.ascontiguousarray(x[bi].reshape(DIM, N)).astype(NPBF16)
        # xt[qi, p, a, c] = x.T[qi*256 + a*128 + p, c]
        xt = np.ascontiguousarray(
            xb.T.reshape(NQ, NT // NQ, P, DIM).transpose(0, 2, 1, 3)
        )
        in_maps.append({"x": xb, "xt": xt, "wkqo": wkqo, "wv": wv_pk, "b": b})
    return in_maps


_NC_CACHE = {}


def get_program():
    if "nc" not in _NC_CACHE:
        _NC_CACHE["nc"] = build_program()
    return _NC_CACHE["nc"]


def kernel(x, w_qkv, w_out, b_out, **_unused):
    nc = get_program()
    in_maps = prep_inputs(x, w_qkv, w_out, b_out)
    res = run_bass_kernel_spmd(nc, in_maps, list(range(N_CORES)))
    y = np.stack(
        [np.asarray(res.results[c]["y"]).astype(np.float32) for c in range(N_CORES)],
        axis=0,
    )
    return y.reshape(B, DIM, H, W)


# revision 5
# speedup vs baseline: 1.0860x; 1.0860x over previous
"""Multi-head channel-attention kernel for Trainium2 (8 NeuronCores, SPMD).

Reference computation (per batch b, x = [256, N] with N = 64*64 = 4096):
    qkv   = w_qkv @ x
    q,k,v = per-head [256, N] slices of qkv
    logit = (q*scale) @ k.T          # [256, 256] (contraction over N)
    wts   = softmax(logit, -1)
    out_h = wts @ v
    y     = w_out @ stack_h(out_h) + b_out

Distribution: pure data-parallel — batch 8 across 8 cores, one batch per
core, no collectives.

The kernel exploits that attention is over the *channel* axis (n >> c):

    logit_h = (Wq_h * scale) @ (x @ x.T) @ Wk_h.T
    y       = (sum_h W_h @ softmax_h @ Wv_h) @ x + b  =  Wstar @ x + b

so the only n-wide work is the Gram matrix G = x @ x.T (one pass over x)
and the final Wstar @ x (second pass). Everything else is [256,256]-sized.
Per-batch FLOPs drop from 12.9G (direct) to 1.6G.

Pipeline (all matmuls TensorE, bf16 operands, fp32 PSUM):
    G    = xT.T @ xT                  (xT shipped pre-transposed from host)
    A_h  = G @ Wk_h.T                 (uses G's symmetry: lhsT = G)
    L_h  = (Wq_h*scale) @ A_h         -> PSUM
    E_h  = exp(L_h) on ScalarE straight from PSUM, accum_out = row sums;
           row-normalize with VectorE reciprocal (softmax; logits are O(1)
           for this problem so no max-subtraction is needed)
    M_hT = E_h-contraction with WoT   (computed directly transposed:
           lhsT = Ehat, rhs = WoT — no on-chip transposes anywhere)
    WstarT = sum_h Wv_h-contraction with M_hT
    y    = WstarT.T @ x + b           (bias folded into the PSUM drains)

The four [256,256]-per-head stages are software-pipelined across heads
(emission order A0 A1 L0 A2 L1 M0 A3 L2 M1 L3 M2 M3 Wst) so the PE never
waits on the softmax chain of the head in flight.

DMA schedule: the per-core HBM link (~330 GB/s) is saturated for the
first ~20us, so transfers are chunked (always keeping >=2KB per
partition line — shorter lines halve DMA throughput) and ordered by
first use, with triggers alternating across the SP and ACT HWDGE
queues: xt slabs first (G consumes them incrementally), then per-k
weight chunks split [wk|wq interleaved by head-pair] -> wo -> wv, then
x in per-k quarters. Output y is written in bf16 (host converts back to
fp32; quantization adds <0.4% relative error, well inside the 2e-2
gate) as 8 chunk-pair stores issued as soon as each pair's PSUM drains
complete, so the store stream overlaps the tail of the final GEMM
instead of serializing after it. The final GEMM is emitted ot-outer /
k-mid / j-inner so each [128,128] stationary block of WstarT streams 8
chunks of 512 columns, and the 8 PSUM banks hold a full ot pass.
"""

import numpy as np
import ml_dtypes

import concourse.bass as bass
import concourse.mybir as mybir
import concourse.tile as tile
from concourse.bass import ts
from concourse.bass_utils import run_bass_kernel_spmd
from concourse.vector_clock import ScopedClock

B, DIM, H, W = 8, 256, 64, 64
HEADS = 4
N = H * W            # 4096
P = 128
KT = DIM // P        # 2 channel tiles
NT = N // P          # 32 n-tiles of 128
NQ = 8               # xT shipped in 8 slabs of 4 n-tiles (2KB lines)
NCH = N // 512       # 8 n-chunks of 512
N_CORES = 8

F32 = mybir.dt.float32
BF16 = mybir.dt.bfloat16
NPBF16 = ml_dtypes.bfloat16


def _split_multi_waits(nc, max_waits=1):
    """The walrus build in this container rejects instructions carrying more
    than one sync-wait. Move excess waits onto same-engine carrier NOPs
    inserted immediately before the instruction (engines are in-order, so
    waiting earlier on the same stream is equivalent)."""
    n_split = 0
    for f in nc.m.functions:
        for bb in f.blocks:
            old = list(bb.instructions)
            new = []
            changed = False
            for inst in old:
                si = inst.sync_info
                waits = list(si.on_wait) if si and si.on_wait else []
                if len(waits) > max_waits:
                    changed = True
                    for w in waits[max_waits:]:
                        n_split += 1
                        new.append(
                            mybir.InstNoOp(
                                name=f"wsplit_{n_split}_{inst.name}",
                                engine=inst.engine,
                                ins=[],
                                outs=[],
                                sync_info=mybir.SyncInfo(on_wait=[w], on_update=[]),
                            )
                        )
                    inst.sync_info = mybir.SyncInfo(
                        on_wait=waits[:max_waits], on_update=si.on_update
                    )
                new.append(inst)
            if changed:
                bb.instructions = new
    return n_split


def _minimal_exit(self, tick_clock, wait_clock):
    """TileContext._drain_and_barrier replacement: one SP drain carrying the
    global-clock waits (split onto NOPs by _split_multi_waits afterwards).

    The stock exit adds two all-engine barriers and ~200 per-semaphore
    clears (~10 us). They are redundant here: the bass preamble range-clears
    the whole kernel semaphore range at startup, and bass's own postamble
    still drains every engine.
    """
    nc = self.nc
    drain = nc.sync.drain()
    wait_clock.add_sem_waits(drain.ins, ScopedClock({None: tick_clock.global_clock}))
    popped = nc._tile_sem_poison_stack.pop()
    assert popped is self._sem_poison


def build_program():
    """Build the single-core Bass program (run SPMD across 8 cores)."""
    nc = bass.Bass()

    x_d = nc.declare_dram_parameter("x", [DIM, N], BF16, isOutput=False)
    # xt: [NQ][128, 4, 256]; slab qi, element (p, a, c) = x.T[qi*512 + a*128 + p, c]
    xt_d = nc.declare_dram_parameter("xt", [NQ, P, NT // NQ, DIM], BF16, isOutput=False)
    # wkq[k][half] = [128, 1024]: [wk_h|wq_h for h in (2*half, 2*half+1)]
    wkq_d = nc.declare_dram_parameter("wkq", [KT, 2, P, 4 * DIM], BF16, isOutput=False)
    # wo[k] = [128, 1024]: woT head-concat; wv[k] = [128, 1024]: wv head-concat
    wo_d = nc.declare_dram_parameter("wo", [KT, P, HEADS * DIM], BF16, isOutput=False)
    wv_d = nc.declare_dram_parameter("wv", [KT, P, HEADS * DIM], BF16, isOutput=False)
    b_d = nc.declare_dram_parameter("b", [DIM, 1], F32, isOutput=False)
    y_d = nc.declare_dram_parameter("y", [DIM, N], BF16, isOutput=True)

    prev_exit = tile.TileContext._drain_and_barrier
    tile.TileContext._drain_and_barrier = _minimal_exit
    try:
        _build_body(nc, tc_args=(x_d, xt_d, wkq_d, wo_d, wv_d, b_d, y_d))
    finally:
        tile.TileContext._drain_and_barrier = prev_exit

    # NOTE: hoisting startup work before the init barrier was tried and lost
    # time — the runtime preamble (~6.5us) gates all engines anyway, and
    # pre-barrier work just delays the barrier release for everyone.
    _split_multi_waits(nc)
    return nc


def _build_body(nc, tc_args):
    x_d, xt_d, wkq_d, wo_d, wv_d, b_d, y_d = tc_args
    with tile.TileContext(nc) as tc:
        with (
            tc.tile_pool(name="wpool", bufs=1) as wpool,
            tc.tile_pool(name="spool", bufs=2) as spool,
            tc.tile_pool(name="ypool", bufs=2) as ypool,
            tc.tile_pool(name="psum", bufs=1, space="PSUM") as psum,
        ):
            # ---- PE warmup: dummy matmuls during the input DMAs release
            # the HAM clock-gate so G runs at 2.4 GHz from its first
            # instruction; sized to end ~when the first xt slab lands.
            warm = wpool.tile([P, P], BF16, tag="warm")
            nc.gpsimd.memset(warm[:], 0)
            wps = psum.tile([P, P], F32, tag="g0", bufs=1)
            for _ in range(24):
                nc.tensor.matmul(wps[:], warm[:], warm[:], start=True, stop=True)

            # ---- SBUF tiles ----
            xt_sb = [None] * NQ
            wkq_sb = {}
            wo_sb = [None] * KT
            wv_sb = [None] * KT
            b_sb = [None] * KT
            x_sb = {}
            for qi in range(NQ):
                xt_sb[qi] = wpool.tile([P, NT // NQ, DIM], BF16, tag=f"xt{qi}",
                                       name=f"xt{qi}")
            for k in range(KT):
                for hp in range(2):
                    wkq_sb[(k, hp)] = wpool.tile([P, 4 * DIM], BF16,
                                                 tag=f"wkq{k}_{hp}",
                                                 name=f"wkq{k}_{hp}")
                wo_sb[k] = wpool.tile([P, HEADS * DIM], BF16, tag=f"wo{k}",
                                      name=f"wo{k}")
                wv_sb[k] = wpool.tile([P, HEADS * DIM], BF16, tag=f"wv{k}",
                                      name=f"wv{k}")
                b_sb[k] = wpool.tile([P, 1], F32, tag=f"b{k}", name=f"b{k}")
                for q in range(4):
                    x_sb[(k, q)] = wpool.tile([P, 1024], BF16, tag=f"x{k}_{q}",
                                              name=f"x{k}_{q}")

            # ---- load triggers, in first-use order; one k-tile per HWDGE
            # queue so the two queues drain in parallel. x quarters are
            # j-interleaved across the queues so the final GEMM's k0 pass
            # gets x[0] from both queues at once.
            for eng_id, eng in ((0, nc.sync), (1, nc.scalar)):
                k = eng_id
                for qi in range(eng_id, NQ, 2):
                    eng.dma_start(xt_sb[qi][:], xt_d[qi])
                eng.dma_start(wkq_sb[(k, 0)][:], wkq_d[k, 0])
                eng.dma_start(wkq_sb[(k, 1)][:], wkq_d[k, 1])
                eng.dma_start(wo_sb[k][:], wo_d[k])
                eng.dma_start(wv_sb[k][:], wv_d[k])
                eng.dma_start(b_sb[k][:], b_d[ts(k, P), :])
            for q in range(4):
                for k in range(KT):
                    eng = nc.sync if (q + k) % 2 == 0 else nc.scalar
                    eng.dma_start(x_sb[(k, q)][:], x_d[ts(k, P), ts(q, 1024)])

            # ---- G = x @ x.T (fp32 PSUM, 32 accumulation steps) ----------
            g_ps = []
            for ct in range(KT):
                gp = psum.tile([P, DIM], F32, tag=f"g{ct}", bufs=1)
                g_ps.append(gp)
            for i in range(NT):
                qi, a = divmod(i, NT // NQ)
                for ct in range(KT):
                    nc.tensor.matmul(
                        g_ps[ct][:],
                        xt_sb[qi][:, a, ts(ct, P)],
                        xt_sb[qi][:, a, :],
                        start=(i == 0),
                        stop=(i == NT - 1),
                    )
            g_sb = []
            for ct in range(KT):
                g = spool.tile([P, DIM], BF16, tag=f"gs{ct}", bufs=1, name=f"g{ct}")
                nc.any.tensor_copy(g[:], g_ps[ct][:])
                g_sb.append(g)

            # ---- per-head stages, software-pipelined across heads --------
            # stage A(h): A = G @ Wk_h.T          (PE + drain)
            # stage L(h): L = (Wq_h*scale) @ A    (PE -> PSUM) + softmax
            # stage M(h): M_hT = Ehat . WoT       (PE + drain)
            a_all, es_all, lp_all = {}, {}, {}
            m_sb = {}

            def wk_slice(k, h):
                return wkq_sb[(k, h // 2)][:, (h % 2) * 2 * DIM : (h % 2) * 2 * DIM + DIM]

            def wq_slice(k, h, ct):
                o = (h % 2) * 2 * DIM + DIM + ct * P
                return wkq_sb[(k, h // 2)][:, o : o + P]

            def stage_A(h):
                a_sb = []
                for ct in range(KT):
                    ap = psum.tile([P, DIM], F32, tag="a", bufs=2, name=f"ap{h}_{ct}")
                    for k in range(KT):
                        # A[c', d] = sum_c'' G[c'', c'] wkT[c'', d]  (G symmetric)
                        nc.tensor.matmul(
                            ap[:],
                            g_sb[k][:, ts(ct, P)],
                            wk_slice(k, h),
                            start=(k == 0),
                            stop=(k == KT - 1),
                        )
                    at = spool.tile([P, DIM], BF16, tag=f"a{ct}", name=f"at{h}_{ct}")
                    nc.any.tensor_copy(at[:], ap[:])
                    a_sb.append(at)
                a_all[h] = a_sb

            def stage_L(h):
                pl = []
                for ct in range(KT):
                    lp = psum.tile([P, DIM], F32, tag=f"l{ct}", bufs=1, name=f"lp{h}_{ct}")
                    for k in range(KT):
                        # L[c, d] = sum_c' wqT[c', c] A[c', d]
                        nc.tensor.matmul(
                            lp[:],
                            wq_slice(k, h, ct),
                            a_all[h][k][:],
                            start=(k == 0),
                            stop=(k == KT - 1),
                        )
                    pl.append(lp)
                lp_all[h] = pl
                # softmax immediately (ACT/DVE; doesn't occupy the PE)
                es = []
                for ct in range(KT):
                    e = spool.tile([P, DIM], BF16, tag=f"e{ct}", name=f"e{h}_{ct}")
                    s = spool.tile([P, 1], F32, tag=f"s{ct}", name=f"s{h}_{ct}")
                    r = spool.tile([P, 1], F32, tag=f"r{ct}", name=f"r{h}_{ct}")
                    nc.scalar.activation(
                        e[:], pl[ct][:], mybir.ActivationFunctionType.Exp,
                        accum_out=s[:],
                    )
                    nc.vector.reciprocal(r[:], s[:])
                    nc.any.tensor_scalar_mul(e[:], e[:], r[:])
                    es.append(e)
                es_all[h] = es

            def stage_M(h):
                es = es_all[h]
                for dt2 in range(KT):
                    pm = psum.tile([P, DIM], F32, tag="m", bufs=2, name=f"pm{h}_{dt2}")
                    for ct in range(KT):
                        # M_hT[d, o] = sum_c Ehat[c, d] woT[c, o]
                        nc.tensor.matmul(
                            pm[:],
                            es[ct][:, ts(dt2, P)],
                            wo_sb[ct][:, h * DIM : (h + 1) * DIM],
                            start=(ct == 0),
                            stop=(ct == KT - 1),
                        )
                    mt = spool.tile([P, DIM], BF16, tag=f"m{h}_{dt2}", bufs=1,
                                    name=f"mt{h}_{dt2}")
                    m_sb[(h, dt2)] = mt
                    nc.any.tensor_copy(mt[:], pm[:])

            # pipelined emission: PE order A0 A1 L0 A2 L1 M0 A3 L2 M1 L3 M2 M3
            stage_A(0)
            stage_A(1)
            stage_L(0)
            stage_A(2)
            stage_L(1)
            stage_M(0)
            stage_A(3)
            stage_L(2)
            stage_M(1)
            stage_L(3)
            stage_M(2)
            stage_M(3)

            # ---- WstarT[c_in, o] = sum_h sum_d wv[d, c_in] M_hT[d, o] ----
            wst_sb = []
            for ct in range(KT):
                wp = psum.tile([P, DIM], F32, tag=f"l{ct}", bufs=1, name=f"wp{ct}")
                first = True
                for h in range(HEADS):
                    for dt2 in range(KT):
                        nc.tensor.matmul(
                            wp[:],
                            wv_sb[dt2][:, h * DIM + ct * P : h * DIM + (ct + 1) * P],
                            m_sb[(h, dt2)][:],
                            start=first,
                            stop=(h == HEADS - 1 and dt2 == KT - 1),
                        )
                        first = False
                wt = spool.tile([P, DIM], BF16, tag=f"wst{ct}", bufs=1, name=f"wt{ct}")
                nc.any.tensor_copy(wt[:], wp[:])
                wst_sb.append(wt)

            # ---- y = WstarT.T @ x + b ------------------------------------
            # ot-outer / k-mid / j-inner: each stationary [128,128] block of
            # WstarT streams all 8 chunks, using 8 PSUM banks per ot pass.
            # Chunk drains (bias add, bf16 cast) alternate DVE/ACT; each
            # chunk PAIR's store (2KB lines) is issued as soon as both
            # drains land, alternating SP/ACT queues.
            ptags = [("g0", 1), ("g1", 1), ("a", 2), ("a", 2),
                     ("l0", 1), ("l1", 1), ("m", 2), ("m", 2)]
            y_sb = {}
            for ot in range(KT):
                y_sb[ot] = ypool.tile([P, N], BF16, tag=f"y{ot}", bufs=1,
                                      name=f"ysb{ot}")
            for ot in range(KT):
                pys = []
                for k in range(KT):
                    for j in range(NCH):
                        if k == 0:
                            py = psum.tile([P, 512], F32, tag=ptags[j][0],
                                           bufs=ptags[j][1], name=f"py{ot}_{j}")
                            pys.append(py)
                        else:
                            py = pys[j]
                        nc.tensor.matmul(
                            py[:],
                            wst_sb[k][:, ts(ot, P)],
                            x_sb[(k, j // 2)][:, (j % 2) * 512 : (j % 2) * 512 + 512],
                            start=(k == 0),
                            stop=(k == KT - 1),
                        )
                        if k == KT - 1:
                            dst = y_sb[ot][:, ts(j, 512)]
                            if j % 2 == 0:
                                nc.vector.tensor_scalar_add(dst, py[:], b_sb[ot][:])
                            else:
                                nc.scalar.add(dst, py[:], b_sb[ot][:])
                                jp = j // 2
                                seng = nc.sync if (ot * 4 + jp) % 2 == 0 else nc.scalar
                                seng.dma_start(
                                    y_d[ts(ot, P), ts(jp, 1024)],
                                    y_sb[ot][:, ts(jp, 1024)],
                                )


def prep_inputs(x, w_qkv, w_out, b_out):
    """Host-side packing: per-core input dicts (numpy only)."""
    x = np.asarray(x, dtype=np.float32)
    w_qkv = np.asarray(w_qkv, dtype=np.float32)
    w_out = np.asarray(w_out, dtype=np.float32)
    b_out = np.asarray(b_out, dtype=np.float32)

    scale = float(DIM) ** -0.5
    wq = w_qkv[0 * HEADS * DIM : 1 * HEADS * DIM].reshape(HEADS, DIM, DIM)
    wk = w_qkv[1 * HEADS * DIM : 2 * HEADS * DIM].reshape(HEADS, DIM, DIM)
    wv = w_qkv[2 * HEADS * DIM : 3 * HEADS * DIM].reshape(HEADS, DIM, DIM)

    # wqT[c', h, c] = wq[h, c, c'] * scale
    wqT = np.transpose(wq, (2, 0, 1)) * scale
    # wkT[c', h, d] = wk[h, d, c']
    wkT = np.transpose(wk, (2, 0, 1))
    # wvn[d, h, c_in] = wv[h, d, c_in]  (natural orientation)
    wvn = np.transpose(wv, (1, 0, 2))
    # woT[c, h, o] = w_out[o, c*HEADS + h]
    woT = w_out.reshape(DIM, DIM, HEADS).transpose(1, 2, 0)

    # wkq[k, half] = [wk_h | wq_h] for h in (2*half, 2*half+1)
    wkq = np.empty((KT, 2, P, 4 * DIM), dtype=NPBF16)
    for k in range(KT):
        rs = slice(k * P, (k + 1) * P)
        for h in range(HEADS):
            half, sub = divmod(h, 2)
            wkq[k, half, :, sub * 2 * DIM : sub * 2 * DIM + DIM] = \
                wkT[rs, h, :].astype(NPBF16)
            wkq[k, half, :, sub * 2 * DIM + DIM : (sub + 1) * 2 * DIM] = \
                wqT[rs, h, :].astype(NPBF16)
    wo_pk = np.ascontiguousarray(
        woT.reshape(DIM, HEADS * DIM).astype(NPBF16).reshape(KT, P, HEADS * DIM)
    )
    wv_pk = np.ascontiguousarray(
        wvn.reshape(DIM, HEADS * DIM).astype(NPBF16).reshape(KT, P, HEADS * DIM)
    )
    b = b_out.reshape(DIM, 1).astype(np.float32)

    in_maps = []
    for bi in range(B):
        xb = np.ascontiguousarray(x[bi].reshape(DIM, N)).astype(NPBF16)
        # xt[qi, p, a, c] = x.T[qi*512 + a*128 + p, c]
        xt = np.ascontiguousarray(
            xb.T.reshape(NQ, NT // NQ, P, DIM).transpose(0, 2, 1, 3)
        )
        in_maps.append({"x": xb, "xt": xt, "wkq": wkq, "wo": wo_pk,
                        "wv": wv_pk, "b": b})
    return in_maps


_NC_CACHE = {}


def get_program():
    if "nc" not in _NC_CACHE:
        _NC_CACHE["nc"] = build_program()
    return _NC_CACHE["nc"]


def kernel(x, w_qkv, w_out, b_out, **_unused):
    nc = get_program()
    in_maps = prep_inputs(x, w_qkv, w_out, b_out)
    res = run_bass_kernel_spmd(nc, in_maps, list(range(N_CORES)))
    y = np.stack(
        [np.asarray(res.results[c]["y"]).astype(np.float32) for c in range(N_CORES)],
        axis=0,
    )
    return y.reshape(B, DIM, H, W)


# revision 8
# speedup vs baseline: 1.2934x; 1.1909x over previous
"""Multi-head channel-attention kernel for Trainium2 (8 NeuronCores, SPMD).

Reference computation (per batch b, x = [256, N] with N = 64*64 = 4096):
    qkv   = w_qkv @ x
    q,k,v = per-head [256, N] slices of qkv
    logit = (q*scale) @ k.T          # [256, 256] (contraction over N)
    wts   = softmax(logit, -1)
    out_h = wts @ v
    y     = w_out @ stack_h(out_h) + b_out

Distribution: pure data-parallel — batch 8 across 8 cores, one batch per
core, no collectives.

The kernel exploits that attention is over the *channel* axis (n >> c):

    logit_h = (Wq_h * scale) @ (x @ x.T) @ Wk_h.T
    y       = (sum_h W_h @ softmax_h @ Wv_h) @ x + b  =  Wstar @ x + b

so the only n-wide work is the Gram matrix G = x @ x.T (one pass over x)
and the final Wstar @ x (second pass). Everything else is [256,256]-sized.
Per-batch FLOPs drop from 12.9G (direct) to 1.6G.

Pipeline (all matmuls TensorE, bf16 operands, fp32 PSUM):
    G    = xT.T @ xT                  (xT shipped pre-transposed from host)
    A_h  = G @ Wk_h.T                 (uses G's symmetry: lhsT = G)
    L_h  = (Wq_h*scale) @ A_h         -> PSUM
    E_h  = exp(L_h) on ScalarE straight from PSUM, accum_out = row sums;
           row-normalize with VectorE reciprocal (softmax; logits are O(1)
           for this problem so no max-subtraction is needed)
    M_hT = E_h-contraction with WoT   (computed directly transposed:
           lhsT = Ehat, rhs = WoT — no on-chip transposes anywhere)
    WstarT = sum_h Wv_h-contraction with M_hT
    y    = WstarT.T @ x               (bias added on the host epilogue —
           shipping a [128,1] bias costs 128 4-byte DMA packets, ~2us of
           queue time, for 512 bytes)

The four [256,256]-per-head stages are software-pipelined across heads
(emission order A0 A1 L0 A2 L1 M0 A3 L2 M1 L3 M2 M3 Wst) so the PE never
waits on the softmax chain of the head in flight.

DMA lessons baked in (measured on HW):
  * each partition line is one DMA packet with a ~15-19ns floor, so
    <4KB lines waste bandwidth (2KB lines -> ~108 GB/s per queue);
  * each dma_start trigger costs ~600ns on its issuing engine and the
    HWDGE queue depth is shallow, so a long trigger list BLOCKS the
    engine (softmax exps were stuck behind pending triggers for ~6us);
  * the first ACTIVATE pays a ~1.3us lazy activation-table load, so a
    dummy exp is issued right after the triggers to preload it.
Therefore: 6 load triggers per engine, >=4KB lines everywhere, ordered
by first use (xt slabs -> wkq -> wo|wv -> x halves), one k-tile per
HWDGE queue. Output y is written in bf16 (host converts to fp32 and
adds the bias; quantization adds <0.4% relative error, well inside the
2e-2 gate) in 4KB-line group stores issued as the chunk drains land,
the last group split across both queues by partition half to shorten
the tail. The final GEMM is emitted ot-outer / k-mid / j-inner so each
[128,128] stationary block of WstarT streams 8 chunks of 512 columns
into the 8 PSUM banks.
"""

import numpy as np
import ml_dtypes

import concourse.bass as bass
import concourse.mybir as mybir
import concourse.tile as tile
from concourse.bass import ts
from concourse.bass_utils import run_bass_kernel_spmd
from concourse.vector_clock import ScopedClock

B, DIM, H, W = 8, 256, 64, 64
HEADS = 4
N = H * W            # 4096
P = 128
KT = DIM // P        # 2 channel tiles
NT = N // P          # 32 n-tiles of 128
NQ = 4               # xT shipped in 4 slabs of 8 n-tiles (4KB lines)
NCH = N // 512       # 8 n-chunks of 512
N_CORES = 8

F32 = mybir.dt.float32
BF16 = mybir.dt.bfloat16
NPBF16 = ml_dtypes.bfloat16


def _split_multi_waits(nc, max_waits=1):
    """The walrus build in this container rejects instructions carrying more
    than one sync-wait. Move excess waits onto same-engine carrier NOPs
    inserted immediately before the instruction (engines are in-order, so
    waiting earlier on the same stream is equivalent)."""
    n_split = 0
    for f in nc.m.functions:
        for bb in f.blocks:
            old = list(bb.instructions)
            new = []
            changed = False
            for inst in old:
                si = inst.sync_info
                waits = list(si.on_wait) if si and si.on_wait else []
                if len(waits) > max_waits:
                    changed = True
                    for w in waits[max_waits:]:
                        n_split += 1
                        new.append(
                            mybir.InstNoOp(
                                name=f"wsplit_{n_split}_{inst.name}",
                                engine=inst.engine,
                                ins=[],
                                outs=[],
                                sync_info=mybir.SyncInfo(on_wait=[w], on_update=[]),
                            )
                        )
                    inst.sync_info = mybir.SyncInfo(
                        on_wait=waits[:max_waits], on_update=si.on_update
                    )
                new.append(inst)
            if changed:
                bb.instructions = new
    return n_split


def _minimal_exit(self, tick_clock, wait_clock):
    """TileContext._drain_and_barrier replacement: one SP drain carrying the
    global-clock waits (split onto NOPs by _split_multi_waits afterwards).

    The stock exit adds two all-engine barriers and ~200 per-semaphore
    clears (~10 us). They are redundant here: the bass preamble range-clears
    the whole kernel semaphore range at startup, and bass's own postamble
    still drains every engine.
    """
    nc = self.nc
    drain = nc.sync.drain()
    wait_clock.add_sem_waits(drain.ins, ScopedClock({None: tick_clock.global_clock}))
    popped = nc._tile_sem_poison_stack.pop()
    assert popped is self._sem_poison


def build_program():
    """Build the single-core Bass program (run SPMD across 8 cores)."""
    nc = bass.Bass()

    x_d = nc.declare_dram_parameter("x", [DIM, N], BF16, isOutput=False)
    # xt: [NQ][128, 8, 256]; slab qi, element (p, a, c) = x.T[qi*1024 + a*128 + p, c]
    xt_d = nc.declare_dram_parameter("xt", [NQ, P, NT // NQ, DIM], BF16, isOutput=False)
    # wkq[k] = [128, 2048]: [wk_h | wq_h] per head; rows k*128:(k+1)*128
    wkq_d = nc.declare_dram_parameter("wkq", [KT, P, 8 * DIM], BF16, isOutput=False)
    # wov[k] = [128, 2048]: [woT head-concat | wv head-concat]
    wov_d = nc.declare_dram_parameter("wov", [KT, P, 2 * HEADS * DIM], BF16,
                                      isOutput=False)
    y_d = nc.declare_dram_parameter("y", [DIM, N], BF16, isOutput=True)

    prev_exit = tile.TileContext._drain_and_barrier
    tile.TileContext._drain_and_barrier = _minimal_exit
    try:
        _build_body(nc, tc_args=(x_d, xt_d, wkq_d, wov_d, y_d))
    finally:
        tile.TileContext._drain_and_barrier = prev_exit

    # NOTE: hoisting startup work before the init barrier was tried and lost
    # time — the runtime preamble (~6.5us) gates all engines anyway, and
    # pre-barrier work just delays the barrier release for everyone.
    _split_multi_waits(nc)
    return nc


def _build_body(nc, tc_args):
    x_d, xt_d, wkq_d, wov_d, y_d = tc_args
    OO_, OV_ = 0, HEADS * DIM
    with tile.TileContext(nc) as tc:
        with (
            tc.tile_pool(name="wpool", bufs=1) as wpool,
            tc.tile_pool(name="spool", bufs=2) as spool,
            tc.tile_pool(name="ypool", bufs=2) as ypool,
            tc.tile_pool(name="psum", bufs=1, space="PSUM") as psum,
        ):
            # ---- PE warmup: dummy matmuls during the input DMAs release
            # the HAM clock-gate so G runs at 2.4 GHz from its first
            # instruction; sized to end ~when the first xt slab lands.
            warm = wpool.tile([P, P], BF16, tag="warm")
            nc.gpsimd.memset(warm[:], 0)
            wps = psum.tile([P, P], F32, tag="g0", bufs=1)
            for _ in range(24):
                nc.tensor.matmul(wps[:], warm[:], warm[:], start=True, stop=True)

            # ---- SBUF tiles ----
            xt_sb = [None] * NQ
            wkq_sb = [None] * KT
            wov_sb = [None] * KT
            x_sb = {}
            for qi in range(NQ):
                xt_sb[qi] = wpool.tile([P, NT // NQ, DIM], BF16, tag=f"xt{qi}",
                                       name=f"xt{qi}")
            for k in range(KT):
                wkq_sb[k] = wpool.tile([P, 8 * DIM], BF16, tag=f"wkq{k}",
                                       name=f"wkq{k}")
                wov_sb[k] = wpool.tile([P, 2 * HEADS * DIM], BF16, tag=f"wov{k}",
                                       name=f"wov{k}")
                for hf in range(2):
                    x_sb[(k, hf)] = wpool.tile([P, N // 2], BF16, tag=f"x{k}_{hf}",
                                               name=f"x{k}_{hf}")

            # ---- load triggers, in first-use order; one k-tile per HWDGE
            # queue so the two queues drain in parallel. x halves are
            # split so each k-pass of the final GEMM reads one queue.
            for eng_id, eng in ((0, nc.sync), (1, nc.scalar)):
                k = eng_id
                for qi in range(eng_id, NQ, 2):
                    eng.dma_start(xt_sb[qi][:], xt_d[qi])
                eng.dma_start(wkq_sb[k][:], wkq_d[k])
                eng.dma_start(wov_sb[k][:], wov_d[k])
                eng.dma_start(x_sb[(k, 0)][:], x_d[ts(k, P), 0 : N // 2])
                eng.dma_start(x_sb[(k, 1)][:], x_d[ts(k, P), N // 2 : N])

            # preload the ACT activation table (lazy ~1.3us on first
            # ACTIVATE) while the input DMAs stream
            dumin = spool.tile([P, 1], F32, tag="dumin", name="dumin")
            dume = spool.tile([P, 1], F32, tag="dume", name="dume")
            dums = spool.tile([P, 1], F32, tag="dums", name="dums")
            nc.gpsimd.memset(dumin[:], 0)
            nc.scalar.activation(
                dume[:], dumin[:], mybir.ActivationFunctionType.Exp,
                accum_out=dums[:],
            )

            # ---- G = x @ x.T (fp32 PSUM, 32 accumulation steps) ----------
            g_ps = []
            for ct in range(KT):
                gp = psum.tile([P, DIM], F32, tag=f"g{ct}", bufs=1)
                g_ps.append(gp)
            for i in range(NT):
                qi, a = divmod(i, NT // NQ)
                for ct in range(KT):
                    nc.tensor.matmul(
                        g_ps[ct][:],
                        xt_sb[qi][:, a, ts(ct, P)],
                        xt_sb[qi][:, a, :],
                        start=(i == 0),
                        stop=(i == NT - 1),
                    )
            g_sb = []
            for ct in range(KT):
                g = spool.tile([P, DIM], BF16, tag=f"gs{ct}", bufs=1, name=f"g{ct}")
                nc.any.tensor_copy(g[:], g_ps[ct][:])
                g_sb.append(g)

            # ---- per-head stages, software-pipelined across heads --------
            # stage A(h): A = G @ Wk_h.T          (PE + drain)
            # stage L(h): L = (Wq_h*scale) @ A    (PE -> PSUM) + softmax
            # stage M(h): M_hT = Ehat . WoT       (PE + drain)
            a_all, es_all, lp_all = {}, {}, {}
            m_sb = {}

            def stage_A(h):
                a_sb = []
                for ct in range(KT):
                    ap = psum.tile([P, DIM], F32, tag="a", bufs=2, name=f"ap{h}_{ct}")
                    for k in range(KT):
                        # A[c', d] = sum_c'' G[c'', c'] wkT[c'', d]  (G symmetric)
                        nc.tensor.matmul(
                            ap[:],
                            g_sb[k][:, ts(ct, P)],
                            wkq_sb[k][:, h * 2 * DIM : h * 2 * DIM + DIM],
                            start=(k == 0),
                            stop=(k == KT - 1),
                        )
                    at = spool.tile([P, DIM], BF16, tag=f"a{ct}", name=f"at{h}_{ct}")
                    nc.any.tensor_copy(at[:], ap[:])
                    a_sb.append(at)
                a_all[h] = a_sb

            def stage_L(h):
                pl = []
                for ct in range(KT):
                    lp = psum.tile([P, DIM], F32, tag=f"l{ct}", bufs=1, name=f"lp{h}_{ct}")
                    for k in range(KT):
                        # L[c, d] = sum_c' wqT[c', c] A[c', d]
                        o = h * 2 * DIM + DIM + ct * P
                        nc.tensor.matmul(
                            lp[:],
                            wkq_sb[k][:, o : o + P],
                            a_all[h][k][:],
                            start=(k == 0),
                            stop=(k == KT - 1),
                        )
                    pl.append(lp)
                lp_all[h] = pl
                # softmax immediately (ACT/DVE; doesn't occupy the PE)
                es = []
                for ct in range(KT):
                    e = spool.tile([P, DIM], BF16, tag=f"e{ct}", name=f"e{h}_{ct}")
                    s = spool.tile([P, 1], F32, tag=f"s{ct}", name=f"s{h}_{ct}")
                    r = spool.tile([P, 1], F32, tag=f"r{ct}", name=f"r{h}_{ct}")
                    nc.scalar.activation(
                        e[:], pl[ct][:], mybir.ActivationFunctionType.Exp,
                        accum_out=s[:],
                    )
                    nc.vector.reciprocal(r[:], s[:])
                    nc.any.tensor_scalar_mul(e[:], e[:], r[:])
                    es.append(e)
                es_all[h] = es

            def stage_M(h):
                es = es_all[h]
                for dt2 in range(KT):
                    pm = psum.tile([P, DIM], F32, tag="m", bufs=2, name=f"pm{h}_{dt2}")
                    for ct in range(KT):
                        # M_hT[d, o] = sum_c Ehat[c, d] woT[c, o]
                        nc.tensor.matmul(
                            pm[:],
                            es[ct][:, ts(dt2, P)],
                            wov_sb[ct][:, OO_ + h * DIM : OO_ + (h + 1) * DIM],
                            start=(ct == 0),
                            stop=(ct == KT - 1),
                        )
                    mt = spool.tile([P, DIM], BF16, tag=f"m{h}_{dt2}", bufs=1,
                                    name=f"mt{h}_{dt2}")
                    m_sb[(h, dt2)] = mt
                    nc.any.tensor_copy(mt[:], pm[:])

            # pipelined emission: PE order A0 A1 L0 A2 L1 M0 A3 L2 M1 L3 M2 M3
            stage_A(0)
            stage_A(1)
            stage_L(0)
            stage_A(2)
            stage_L(1)
            stage_M(0)
            stage_A(3)
            stage_L(2)
            stage_M(1)
            stage_L(3)
            stage_M(2)
            stage_M(3)

            # ---- WstarT[c_in, o] = sum_h sum_d wv[d, c_in] M_hT[d, o] ----
            wst_sb = []
            for ct in range(KT):
                wp = psum.tile([P, DIM], F32, tag=f"l{ct}", bufs=1, name=f"wp{ct}")
                first = True
                for h in range(HEADS):
                    for dt2 in range(KT):
                        nc.tensor.matmul(
                            wp[:],
                            wov_sb[dt2][:, OV_ + h * DIM + ct * P : OV_ + h * DIM + (ct + 1) * P],
                            m_sb[(h, dt2)][:],
                            start=first,
                            stop=(h == HEADS - 1 and dt2 == KT - 1),
                        )
                        first = False
                wt = spool.tile([P, DIM], BF16, tag=f"wst{ct}", bufs=1, name=f"wt{ct}")
                nc.any.tensor_copy(wt[:], wp[:])
                wst_sb.append(wt)

            # ---- y = WstarT.T @ x ----------------------------------------
            # ot-outer / k-mid / j-inner: each stationary [128,128] block of
            # WstarT streams all 8 chunks, using 8 PSUM banks per ot pass.
            # Chunk drains (bf16 cast) alternate DVE/GpSimd (ACT stays free
            # for store triggers); stores go out in 4KB-line groups as the
            # drains land, the last group split by partition half across
            # both queues to shorten the tail.
            ptags = [("g0", 1), ("g1", 1), ("a", 2), ("a", 2),
                     ("l0", 1), ("l1", 1), ("m", 2), ("m", 2)]
            y_sb = {}
            for ot in range(KT):
                y_sb[ot] = ypool.tile([P, N], BF16, tag=f"y{ot}", bufs=1,
                                      name=f"ysb{ot}")
            for ot in range(KT):
                pys = []
                for k in range(KT):
                    for j in range(NCH):
                        if k == 0:
                            py = psum.tile([P, 512], F32, tag=ptags[j][0],
                                           bufs=ptags[j][1], name=f"py{ot}_{j}")
                            pys.append(py)
                        else:
                            py = pys[j]
                        nc.tensor.matmul(
                            py[:],
                            wst_sb[k][:, ts(ot, P)],
                            x_sb[(k, j // 4)][:, (j % 4) * 512 : (j % 4) * 512 + 512],
                            start=(k == 0),
                            stop=(k == KT - 1),
                        )
                        if k == KT - 1:
                            dst = y_sb[ot][:, ts(j, 512)]
                            if j % 2 == 0:
                                nc.vector.tensor_copy(dst, py[:])
                            else:
                                nc.scalar.add(dst, py[:], 0.0)
                            if j == 3:
                                eng = nc.sync if ot == 0 else nc.scalar
                                eng.dma_start(y_d[ts(ot, P), 0 : N // 2],
                                              y_sb[ot][:, 0 : N // 2])
                            elif j == 7:
                                if ot == 0:
                                    nc.sync.dma_start(
                                        y_d[ts(ot, P), N // 2 : N],
                                        y_sb[ot][:, N // 2 : N])
                                else:
                                    # split by partition half across both
                                    # queues: halves the tail store time
                                    nc.sync.dma_start(
                                        y_d[ot * P : ot * P + 64, N // 2 : N],
                                        y_sb[ot][0:64, N // 2 : N])
                                    nc.scalar.dma_start(
                                        y_d[ot * P + 64 : ot * P + P, N // 2 : N],
                                        y_sb[ot][64:P, N // 2 : N])


def prep_inputs(x, w_qkv, w_out, b_out):
    """Host-side packing: per-core input dicts (numpy only)."""
    x = np.asarray(x, dtype=np.float32)
    w_qkv = np.asarray(w_qkv, dtype=np.float32)
    w_out = np.asarray(w_out, dtype=np.float32)

    scale = float(DIM) ** -0.5
    wq = w_qkv[0 * HEADS * DIM : 1 * HEADS * DIM].reshape(HEADS, DIM, DIM)
    wk = w_qkv[1 * HEADS * DIM : 2 * HEADS * DIM].reshape(HEADS, DIM, DIM)
    wv = w_qkv[2 * HEADS * DIM : 3 * HEADS * DIM].reshape(HEADS, DIM, DIM)

    # wqT[c', h, c] = wq[h, c, c'] * scale
    wqT = np.transpose(wq, (2, 0, 1)) * scale
    # wkT[c', h, d] = wk[h, d, c']
    wkT = np.transpose(wk, (2, 0, 1))
    # wvn[d, h, c_in] = wv[h, d, c_in]  (natural orientation)
    wvn = np.transpose(wv, (1, 0, 2))
    # woT[c, h, o] = w_out[o, c*HEADS + h]
    woT = w_out.reshape(DIM, DIM, HEADS).transpose(1, 2, 0)

    # wkq[k] = [wk_h | wq_h] per head
    wkq = np.empty((KT, P, 8 * DIM), dtype=NPBF16)
    for k in range(KT):
        rs = slice(k * P, (k + 1) * P)
        for h in range(HEADS):
            wkq[k, :, h * 2 * DIM : h * 2 * DIM + DIM] = wkT[rs, h, :].astype(NPBF16)
            wkq[k, :, h * 2 * DIM + DIM : (h + 1) * 2 * DIM] = \
                wqT[rs, h, :].astype(NPBF16)
    # wov[k] = [woT head-concat | wv head-concat]
    wov = np.empty((KT, P, 2 * HEADS * DIM), dtype=NPBF16)
    wov[:, :, 0 : HEADS * DIM] = \
        woT.reshape(DIM, HEADS * DIM).astype(NPBF16).reshape(KT, P, HEADS * DIM)
    wov[:, :, HEADS * DIM : 2 * HEADS * DIM] = \
        wvn.reshape(DIM, HEADS * DIM).astype(NPBF16).reshape(KT, P, HEADS * DIM)

    in_maps = []
    for bi in range(B):
        xb = np.ascontiguousarray(x[bi].reshape(DIM, N)).astype(NPBF16)
        # xt[qi, p, a, c] = x.T[qi*1024 + a*128 + p, c]
        xt = np.ascontiguousarray(
            xb.T.reshape(NQ, NT // NQ, P, DIM).transpose(0, 2, 1, 3)
        )
        in_maps.append({"x": xb, "xt": xt, "wkq": wkq, "wov": wov})
    return in_maps


_NC_CACHE = {}


def get_program():
    if "nc" not in _NC_CACHE:
        _NC_CACHE["nc"] = build_program()
    return _NC_CACHE["nc"]


def kernel(x, w_qkv, w_out, b_out, **_unused):
    nc = get_program()
    in_maps = prep_inputs(x, w_qkv, w_out, b_out)
    res = run_bass_kernel_spmd(nc, in_maps, list(range(N_CORES)))
    b_out = np.asarray(b_out, dtype=np.float32)
    y = np.stack(
        [np.asarray(res.results[c]["y"]).astype(np.float32) for c in range(N_CORES)],
        axis=0,
    )
    y += b_out[None, :, None]
    return y.reshape(B, DIM, H, W)
